# revision 39
# baseline (speedup 1.0000x reference)
"""Trainium2 Bass kernel: prototypical-network cosine CE loss.

reference math:
  sp = supp.mean(axis=-2)            # [5, 2048]   (supp [1,5,64,2048])
  qp = query.mean(axis=-2)           # [512, 2048] (query [1,512,64,2048])
  dist[q,s] = <qp[q], sp[s]> / max(|qp[q]|*|sp[s]|, 1e-8)
  loss = mean_q(-log_softmax(dist)[q, y_q])
  returns (loss, 1 - dist)

Distribution: data-parallel over the 512 queries -> 64 per core on 8 cores,
supp replicated.  Each core streams its 33.5MB query shard from HBM once;
the T-mean is done on the TensorEngine with a block-diagonal ones/64
stationary (reduces the partition axis at 1 column/cycle in fp32r).  The
tiny [64,5] epilogue (cosine, log-softmax CE) runs per core; host gathers
dist shards and averages the per-query CE.
"""

import numpy as np

import concourse.bass as bass
import concourse.mybir as mybir
from concourse import bacc, tile
from concourse.bass_utils import run_bass_kernel_spmd

F32 = mybir.dt.float32
F32R = mybir.dt.float32r
AX = mybir.AxisListType
AF = mybir.ActivationFunctionType
OP = mybir.AluOpType

N_CORES = 8
S, Q, T, D = 5, 512, 64, 2048
QL = Q // N_CORES              # 64 queries per core
N_BLK = 8                      # query stream blocks per core
CW = 64 + 25 + 128 + 5        # consts columns: bd64 | bdsup | ident | onehot


def _build_nc() -> bass.Bass:
    # Bacc (not raw Bass): its compile() pass splits multi-semaphore waits
    # into InstEventSemaphore pairs — walrus codegen allows only one sync
    # wait per instruction.
    nc = bacc.Bacc(None, target_bir_lowering=False)

    supp_d = nc.declare_dram_parameter("supp", [S, T, D], F32, isOutput=False)
    query_d = nc.declare_dram_parameter("query", [QL, T, D], F32, isOutput=False)
    # all small constants ride in ONE param -> ONE DMA -> ONE semaphore, so
    # the fp32r self-loading matmuls (which allow only a single sync wait)
    # never need a second wait for their stationary operand.
    consts_d = nc.declare_dram_parameter("consts", [128, CW], F32, isOutput=False)
    dist_out_d = nc.declare_dram_parameter("out_dist", [QL, S], F32, isOutput=True)
    ce_out_d = nc.declare_dram_parameter("out_ce", [QL, 1], F32, isOutput=True)

    NCH = D // 128             # 16 transpose chunks of 128
    with tile.TileContext(nc) as tc:
        with (
            tc.tile_pool(name="consts", bufs=1) as cpool,
            tc.tile_pool(name="stream", bufs=1) as qpool,
            tc.tile_pool(name="persist", bufs=1) as ppool,
            tc.tile_pool(name="small", bufs=1) as spool,
            tc.tile_pool(name="psum_acc", bufs=1, space="PSUM") as papool,
            tc.tile_pool(name="psum_small", bufs=3, space="PSUM") as pspool,
        ):
            # ---- constants (one DMA) ----
            # The tile is fp32r so the fp32r matmul stationaries (bd64, bdsup)
            # satisfy the verifier's rounded-producer rule; identity/onehot
            # hold 0/1 values (fp32r-invariant) and are bitcast back to f32.
            consts_sb = cpool.tile([128, CW], F32R)
            nc.sync.dma_start(out=consts_sb, in_=consts_d[:].bitcast(F32R))
            bd64_r = consts_sb[:, 0:64]
            bdsup_r = consts_sb[:T, 64 : 64 + S * S]
            ident_sb = consts_sb[:, 89:217].bitcast(F32)
            oh_sb = consts_sb[:QL, 217:222].bitcast(F32)

            # warm-up matmul: consumes the consts DMA semaphore on PE before
            # any real matmul, so later matmuls carry only their own wait.
            warm_ps = pspool.tile([64, 64], F32, tag="ps_small")
            nc.tensor.matmul(warm_ps, lhsT=bd64_r, rhs=bd64_r, start=True, stop=True)

            # ---- supp -> sp (mean over T), scaled by 1/sn, transposed ----
            # One [64t, D] tile per s (single contiguous DMA each); stationary
            # column-block s of bdsup selects output row s, so 5 accumulating
            # matmuls land sp[0:5, chunk] at PSUM base partition 0.
            sp_sb = ppool.tile([S, D], F32)
            supp_tiles = []
            for s in range(S):
                stile = qpool.tile([T, D], F32R, tag=f"supp_{s}", bufs=1)
                nc.sync.dma_start(out=stile, in_=supp_d[s].bitcast(F32R))
                supp_tiles.append(stile)
            for c in range(4):
                ps = pspool.tile([S, 512], F32, tag="ps_small")
                for s in range(S):
                    nc.tensor.matmul(
                        ps,
                        lhsT=bdsup_r[:, S * s : S * (s + 1)],
                        rhs=supp_tiles[s][:, 512 * c : 512 * (c + 1)],
                        start=(s == 0),
                        stop=(s == S - 1),
                    )
                nc.scalar.copy(out=sp_sb[:, 512 * c : 512 * (c + 1)], in_=ps)

            # sn2 -> rsn, scale sp by rsn (folds 1/sn into the num matmul)
            snscr = spool.tile([S, D], F32)
            sn2 = spool.tile([S, 1], F32)
            nc.scalar.activation(out=snscr, in_=sp_sb, func=AF.Square, accum_out=sn2)
            sn = spool.tile([S, 1], F32)
            nc.scalar.sqrt(sn, sn2)
            rsn = spool.tile([S, 1], F32)
            nc.vector.reciprocal(rsn, sn)
            # sps on ACT so the spT transposes' data dep and psum-slot WAR
            # land on the same (ACT) semaphore.
            sps = ppool.tile([S, D], F32)
            nc.scalar.activation(out=sps, in_=sp_sb, func=AF.Copy, bias=0.0, scale=rsn)

            spT = ppool.tile([128, S * NCH], F32)  # chunk c at cols [S*c, S*c+S)
            for c in range(NCH):
                pt = pspool.tile([128, S], F32, tag="ps_small")
                nc.tensor.transpose(
                    pt, sps[:, 128 * c : 128 * (c + 1)], ident_sb[:S, :S]
                )
                nc.scalar.copy(out=spT[:, S * c : S * (c + 1)], in_=pt)

            # ---- query stream: T-mean into PSUM [64, 2048] ----
            # partition p = (q_local * 2 + t_half); t = t_half*32 + gg*4 + gin.
            # Stationary bd64[p, m] = 1/64 iff m == p//2 reduces the partition
            # axis per query; 32 accumulating matmuls per 512-wide d-chunk.
            qp_ps = papool.tile([QL, D], F32)  # 4 PSUM banks
            qsrc = query_d[:].rearrange(
                "q (t2 gg gin) d -> gg (q t2) gin d", t2=2, gin=4
            )
            # 1MB tiles with bufs=8: the slot-reuse WAW partner is exactly 8
            # DMAs back, which lands on the SAME DMAHW lane (8-lane global
            # round-robin), so its wait merges with the lane-FIFO wait and
            # each DMA carries <=2 sync waits (the codegen limit).
            for gg in range(N_BLK):
                for gin in range(4):
                    qt = qpool.tile([128, D], F32R, tag="qt", bufs=8)
                    nc.sync.dma_start(
                        out=qt, in_=qsrc[gg][:, gin, :].bitcast(F32R)
                    )
                    for c in range(4):
                        nc.tensor.matmul(
                            qp_ps[:, 512 * c : 512 * (c + 1)],
                            lhsT=bd64_r,
                            rhs=qt[:, 512 * c : 512 * (c + 1)],
                            start=(gg == 0 and gin == 0),
                            stop=(gg == N_BLK - 1 and gin == 3),
                        )

            qp_sb = ppool.tile([QL, D], F32)
            nc.scalar.copy(out=qp_sb, in_=qp_ps)

            # qn2 -> rqn
            qnscr = spool.tile([QL, D], F32)
            qn2 = spool.tile([QL, 1], F32)
            nc.scalar.activation(out=qnscr, in_=qp_sb, func=AF.Square, accum_out=qn2)
            qn = spool.tile([QL, 1], F32)
            nc.scalar.sqrt(qn, qn2)
            rqn = spool.tile([QL, 1], F32)
            nc.vector.reciprocal(rqn, qn)

            # qp^T chunks -> numT = qp @ (sp * rsn)^T
            qpT = ppool.tile([128, 64 * NCH], F32)
            for c in range(NCH):
                pt2 = pspool.tile([128, 64], F32, tag="ps_small")
                nc.tensor.transpose(
                    pt2, qp_sb[:, 128 * c : 128 * (c + 1)], ident_sb[:QL, :QL]
                )
                nc.scalar.copy(out=qpT[:, 64 * c : 64 * (c + 1)], in_=pt2)

            numT_ps = pspool.tile([QL, S], F32, tag="ps_num", bufs=1)
            for c in range(NCH):
                nc.tensor.matmul(
                    numT_ps,
                    lhsT=qpT[:, 64 * c : 64 * (c + 1)],
                    rhs=spT[:, S * c : S * (c + 1)],
                    start=(c == 0),
                    stop=(c == NCH - 1),
                )

            # dist = numT * rqn ; outputs
            dist = spool.tile([QL, S], F32)
            nc.vector.tensor_scalar_mul(dist, numT_ps, rqn)
            om = spool.tile([QL, S], F32)
            nc.scalar.activation(out=om, in_=dist, func=AF.Copy, bias=1.0, scale=-1.0)
            nc.sync.dma_start(out=dist_out_d[:], in_=om)

            # CE: ce_q = max + log(sum(exp(dist - max))) - dist[y_q]
            m = spool.tile([QL, 1], F32)
            nc.vector.reduce_max(m, dist, axis=AX.X)
            negm = spool.tile([QL, 1], F32)
            nc.scalar.mul(negm, m, -1.0)
            e = spool.tile([QL, S], F32)
            sume = spool.tile([QL, 1], F32)
            nc.scalar.activation(
                out=e, in_=dist, func=AF.Exp, bias=negm, scale=1.0, accum_out=sume
            )
            lse = spool.tile([QL, 1], F32)
            nc.scalar.activation(out=lse, in_=sume, func=AF.Ln)
            pickscr = spool.tile([QL, S], F32)
            picked = spool.tile([QL, 1], F32)
            nc.vector.scalar_tensor_tensor(
                out=pickscr, in0=dist, scalar=1.0, in1=oh_sb,
                op0=OP.mult, op1=OP.mult, accum_out=picked,
            )
            ce = spool.tile([QL, 1], F32)
            nc.vector.scalar_tensor_tensor(
                out=ce, in0=lse, scalar=m, in1=picked, op0=OP.add, op1=OP.subtract
            )
            nc.sync.dma_start(out=ce_out_d[:], in_=ce)

    nc.finalize()
    return nc


_NC = None


def _get_nc() -> bass.Bass:
    global _NC
    if _NC is None:
        _NC = _build_nc()
    return _NC


def _round_fp32r(x: np.ndarray) -> np.ndarray:
    """Round fp32 to the fp32r grid (round-to-nearest-even at bit 12) —
    required by the BIR verifier for fp32r matmul operands."""
    u = np.ascontiguousarray(x).view(np.uint32)
    r = u + (0x7FF + ((u >> 12) & 1))
    r &= np.uint32(0xFFFFF000)
    return r.view(np.float32)


def _make_in_maps(supp, query, query_ys):
    supp = _round_fp32r(np.asarray(supp, dtype=np.float32)[0])          # [5,64,2048]
    query = _round_fp32r(np.asarray(query, dtype=np.float32)[0])        # [512,64,2048]
    ys = np.asarray(query_ys).astype(np.int64)

    oh = np.eye(S, dtype=np.float32)[ys]                                # [512, 5]

    in_maps = []
    for i in range(N_CORES):
        consts = np.zeros((128, CW), dtype=np.float32)
        # bd64[p, m] = 1/64 iff m == p//2
        consts[np.arange(128), np.arange(128) // 2] = 1.0 / 64.0
        # bdsup column block s selects output row s: [:, 5s+j] = 1/64 iff j==s
        for s in range(S):
            consts[:T, 64 + S * s + s] = 1.0 / 64.0
        consts[:, 89:217] = np.eye(128, dtype=np.float32)
        consts[:QL, 217:222] = oh[i * QL : (i + 1) * QL]
        in_maps.append(
            {
                "supp": supp,
                "query": np.ascontiguousarray(query[i * QL : (i + 1) * QL]),
                "consts": consts,
            }
        )
    return in_maps


def run(supp, query, query_ys, trace=False, **kw):
    """Runs the SPMD kernel; returns ((loss, one_minus_dist), BassKernelResults)."""
    nc = _get_nc()
    in_maps = _make_in_maps(supp, query, query_ys)
    res = run_bass_kernel_spmd(
        nc, in_maps, core_ids=list(range(N_CORES)), trace=trace, **kw
    )
    dist_full = np.concatenate(
        [np.asarray(res.results[i]["out_dist"]) for i in range(N_CORES)], axis=0
    ).astype(np.float32)
    ce_full = np.concatenate(
        [np.asarray(res.results[i]["out_ce"]).reshape(-1) for i in range(N_CORES)]
    )
    loss = np.float32(ce_full.mean(dtype=np.float64))
    return (loss, dist_full), res


def kernel(supp, query, query_ys):
    (loss, dist_full), _ = run(supp, query, query_ys)
    return (loss, dist_full)
